# revision 1
# baseline (speedup 1.0000x reference)
"""JacobianDeterminantLoss Trainium2 kernel (8-core SPMD).

Math: u [2,3,160,192,160] f32 -> loss = mean(relu(-det(J))) where
J = I + grad(phi), phi_c = u_c * (dim_c-1)/2, gradients are np.gradient
style (central interior, one-sided edges).

Strategy:
- Shard H=192 into 8 chunks of 24 rows (1 halo row each side).
- Host pre-pads ghosts (2*u[0]-u[1] reflection) on all three axes so the
  device formula is a uniform central difference everywhere.
- Per core layout: partitions = (b,d) planes (324 incl ghosts) in 3
  K-tiles of <=128; free = (26 h-rows x 162 w-cols).
- D-derivative on PE: band matrix matmul (fp16) -> PSUM f32.
- H/W-derivatives: free-dim shifted subs (DVE/GPSIMD, fp16).
- det in fp16 elementwise; relu(-det)+sum on ACT with f32 accum.
- Host: mask garbage partition rows, final sum / N.
"""
import sys
import numpy as np

if '/opt/trn_rl_repo' not in sys.path:
    sys.path.insert(0, '/opt/trn_rl_repo')

B, C, D, H, W = 2, 3, 160, 192, 160
N_CORES = 8
HC = H // N_CORES          # 24 output rows per core
RH = HC + 2                # 26 rows incl halo
WG = W + 2                 # 162 cols incl ghosts
PL = 2 * (D + 2)           # 324 planes incl ghosts (b-major)
GROUPS = [(0, 128), (126, 128), (252, 72)]   # (plane offset, K)
BR = 6                     # h-rows per compute block
NBLK = HC // BR            # 8 blocks
SCALES = ((D - 1) / 4.0, (H - 1) / 4.0, (W - 1) / 4.0)  # a_c/2

_prog_cache = {}


def _build_program():
    import concourse.tile as tile
    import concourse.mybir as mybir
    from concourse import bacc

    fp16 = mybir.dt.float16
    f32 = mybir.dt.float32
    AT = mybir.AluOpType
    AF = mybir.ActivationFunctionType

    nc = bacc.Bacc("TRN2", target_bir_lowering=False, debug=False,
                   num_devices=N_CORES)
    slab_in = nc.dram_tensor("slab", [C, PL, RH, WG], f32, kind="ExternalInput")
    band_in = nc.dram_tensor("band", [128, 128], fp16, kind="ExternalInput")
    acc_out = nc.dram_tensor("acc", [len(GROUPS), 128, NBLK], f32,
                             kind="ExternalOutput")

    with tile.TileContext(nc) as tc:
        with tc.tile_pool(name="s_raw", bufs=1) as s_pool, \
             tc.tile_pool(name="p_sc", bufs=2) as p_pool, \
             tc.tile_pool(name="blk", bufs=2) as blk, \
             tc.tile_pool(name="accp", bufs=1) as accp, \
             tc.tile_pool(name="misc", bufs=1) as misc, \
             tc.tile_pool(name="psum", bufs=1, space="PSUM") as psum:
            band = misc.tile([128, 128], fp16)
            nc.sync.dma_start(band[:], band_in[:])

            for gi, (k0, K) in enumerate(GROUPS):
                # ---- load + scale-cast the 3 channels of this K-window ----
                Ps = []
                for c in range(C):
                    S = s_pool.tile([128, RH, WG], f32, tag=f"S{c}")
                    nc.sync.dma_start(S[0:K], slab_in[c, k0:k0 + K])
                    P = p_pool.tile([128, RH, WG], fp16, tag=f"P{c}")
                    nc.scalar.activation(P[0:K], S[0:K], AF.Copy,
                                         scale=float(SCALES[c]))
                    Ps.append(P)
                Px, Py, Pz = Ps
                accg = accp.tile([128, NBLK], f32, tag="accg")

                for b in range(NBLK):
                    r0 = 1 + BR * b
                    # free-dim views of the scaled planes
                    def ctr(P_):  # center rows, real cols
                        return P_[0:K, r0:r0 + BR, 1:1 + W]
                    def up(P_):   # h+1
                        return P_[0:K, r0 + 1:r0 + 1 + BR, 1:1 + W]
                    def dn(P_):   # h-1
                        return P_[0:K, r0 - 1:r0 - 1 + BR, 1:1 + W]
                    def rt(P_):   # w+1
                        return P_[0:K, r0:r0 + BR, 2:2 + W]
                    def lt(P_):   # w-1
                        return P_[0:K, r0:r0 + BR, 0:W]

                    # ---- PE: d-axis diffs -> PSUM f32 (raw, no +1) ----
                    HB = BR // 2  # rows per matmul (one PSUM bank each)
                    mx = psum.tile([128, 2, 512], f32, tag="mx")
                    my = psum.tile([128, 2, 512], f32, tag="my")
                    mz = psum.tile([128, 2, 512], f32, tag="mz")
                    for j, (mt, P_) in enumerate(((mx, Px), (my, Py), (mz, Pz))):
                        for hh in range(2):
                            rj = r0 + HB * hh
                            nc.tensor.matmul(
                                mt[0:K, hh, 0:HB * W],
                                band[0:K, 0:K],
                                P_[0:K, rj:rj + HB, 1:1 + W],
                                start=True, stop=True)
                    dxx_s = blk.tile([128, BR, W], fp16, tag="dxx_s")
                    nc.scalar.activation(dxx_s[0:K], mx[0:K, :, 0:HB * W],
                                         AF.Identity, bias=1.0)
                    my_s = blk.tile([128, BR, W], fp16, tag="my_s")
                    nc.scalar.copy(my_s[0:K], my[0:K, :, 0:HB * W])
                    mz_s = blk.tile([128, BR, W], fp16, tag="mz_s")
                    nc.scalar.copy(mz_s[0:K], mz[0:K, :, 0:HB * W])
                    myv = my_s[0:K]
                    mzv = mz_s[0:K]

                    # ---- H/W diffs (fp16). GPSIMD takes 3, DVE takes 3 ----
                    gy_x = blk.tile([128, BR, W], fp16, tag="gy_x")
                    nc.gpsimd.tensor_sub(gy_x[0:K], up(Px), dn(Px))
                    gy_y = blk.tile([128, BR, W], fp16, tag="gy_y")  # +1 folded
                    nc.vector.scalar_tensor_tensor(gy_y[0:K], up(Py), 1.0,
                                                   dn(Py), AT.add, AT.subtract)
                    gy_z = blk.tile([128, BR, W], fp16, tag="gy_z")
                    nc.gpsimd.tensor_sub(gy_z[0:K], up(Pz), dn(Pz))
                    gz_x = blk.tile([128, BR, W], fp16, tag="gz_x")
                    nc.gpsimd.tensor_sub(gz_x[0:K], rt(Px), lt(Px))
                    gz_y = blk.tile([128, BR, W], fp16, tag="gz_y")
                    nc.vector.tensor_sub(gz_y[0:K], rt(Py), lt(Py))
                    gz_z = blk.tile([128, BR, W], fp16, tag="gz_z")  # +1 folded
                    nc.vector.scalar_tensor_tensor(gz_z[0:K], rt(Pz), 1.0,
                                                   lt(Pz), AT.add, AT.subtract)

                    # J = [[mx+1, gy_x, gz_x],
                    #      [my,   gy_y, gz_y],
                    #      [mz,   gy_z, gz_z]]
                    # det = (mx+1)*c1 + gy_x*c2 + gz_x*c3
                    # c1 = gy_y*gz_z - gz_y*gy_z
                    # c2 = gz_y*mz - my*gz_z
                    # c3 = my*gy_z - gy_y*mz
                    t1 = blk.tile([128, BR, W], fp16, tag="t1")
                    nc.vector.tensor_mul(t1[0:K], gy_y[0:K], gz_z[0:K])
                    t2 = blk.tile([128, BR, W], fp16, tag="t2")
                    nc.gpsimd.tensor_mul(t2[0:K], gz_y[0:K], gy_z[0:K])
                    c1 = blk.tile([128, BR, W], fp16, tag="c1")
                    nc.vector.tensor_sub(c1[0:K], t1[0:K], t2[0:K])
                    u1 = blk.tile([128, BR, W], fp16, tag="u1")
                    nc.vector.tensor_mul(u1[0:K], gz_y[0:K], mzv)
                    u2 = blk.tile([128, BR, W], fp16, tag="u2")
                    nc.vector.tensor_mul(u2[0:K], gz_z[0:K], myv)
                    c2 = blk.tile([128, BR, W], fp16, tag="c2")
                    nc.vector.tensor_sub(c2[0:K], u1[0:K], u2[0:K])
                    v1 = blk.tile([128, BR, W], fp16, tag="v1")
                    nc.vector.tensor_mul(v1[0:K], gy_z[0:K], myv)
                    v2 = blk.tile([128, BR, W], fp16, tag="v2")
                    nc.vector.tensor_mul(v2[0:K], gy_y[0:K], mzv)
                    c3 = blk.tile([128, BR, W], fp16, tag="c3")
                    nc.vector.tensor_sub(c3[0:K], v1[0:K], v2[0:K])
                    d1 = blk.tile([128, BR, W], fp16, tag="d1")
                    nc.vector.tensor_mul(d1[0:K], dxx_s[0:K], c1[0:K])
                    d2 = blk.tile([128, BR, W], fp16, tag="d2")
                    nc.gpsimd.tensor_mul(d2[0:K], gy_x[0:K], c2[0:K])
                    d3 = blk.tile([128, BR, W], fp16, tag="d3")
                    nc.vector.tensor_mul(d3[0:K], gz_x[0:K], c3[0:K])
                    s1 = blk.tile([128, BR, W], fp16, tag="s1")
                    nc.vector.tensor_add(s1[0:K], d1[0:K], d2[0:K])
                    det = blk.tile([128, BR, W], fp16, tag="det")
                    nc.vector.tensor_add(det[0:K], s1[0:K], d3[0:K])

                    trash = blk.tile([128, BR, W], fp16, tag="trash")
                    nc.scalar.activation(trash[0:K], det[0:K], AF.Relu,
                                         scale=-1.0,
                                         accum_out=accg[0:K, b:b + 1])
                nc.sync.dma_start(acc_out[gi], accg[:])
    nc.compile()
    return nc


def _valid_mask():
    """[n_groups, 128] bool — which acc rows are real (b,d) planes."""
    mask = np.zeros((len(GROUPS), 128), dtype=bool)
    for gi, (k0, K) in enumerate(GROUPS):
        for m in range(1, K - 1):
            plane = k0 + m
            r = plane % (D + 2)          # position within a batch segment
            if r != 0 and r != D + 1:    # not a ghost plane
                mask[gi, m] = True
    return mask


def _make_band():
    band = np.zeros((128, 128), dtype=np.float16)
    for m in range(128):
        if m + 1 < 128:
            band[m + 1, m] = 1.0
        if m - 1 >= 0:
            band[m - 1, m] = -1.0
    return band


def _make_slabs(u):
    """u [2,3,160,192,160] f32 -> list of 8 per-core slabs [3,324,26,162]."""
    u = np.ascontiguousarray(u, dtype=np.float32)
    pad = np.empty((B, C, D + 2, H + 2, WG), dtype=np.float32)
    pad[:, :, 1:D + 1, 1:H + 1, 1:W + 1] = u
    # ghost h-rows (from real planes/cols region only; ghosts of ghosts below)
    pad[:, :, 1:D + 1, 0, 1:W + 1] = 2 * u[:, :, :, 0, :] - u[:, :, :, 1, :]
    pad[:, :, 1:D + 1, H + 1, 1:W + 1] = 2 * u[:, :, :, -1, :] - u[:, :, :, -2, :]
    # ghost w-cols (including on ghost h-rows)
    pad[:, :, 1:D + 1, :, 0] = 2 * pad[:, :, 1:D + 1, :, 1] - pad[:, :, 1:D + 1, :, 2]
    pad[:, :, 1:D + 1, :, W + 1] = 2 * pad[:, :, 1:D + 1, :, W] - pad[:, :, 1:D + 1, :, W - 1]
    # ghost planes (fully padded rows/cols)
    pad[:, :, 0] = 2 * pad[:, :, 1] - pad[:, :, 2]
    pad[:, :, D + 1] = 2 * pad[:, :, D] - pad[:, :, D - 1]
    # planes axis b-major: [C, 324, H+2, WG]
    planes = np.concatenate([pad[0], pad[1]], axis=1)
    slabs = []
    for k in range(N_CORES):
        r0 = HC * k  # padded row index of first halo row (= h-1 of chunk)
        slab = np.ascontiguousarray(planes[:, :, r0:r0 + RH, :])
        slabs.append(slab)
    return slabs


def kernel(displacement_field: np.ndarray) -> np.ndarray:
    from concourse.bass_utils import run_bass_kernel_spmd

    if 'nc' not in _prog_cache:
        _prog_cache['nc'] = _build_program()
    nc = _prog_cache['nc']

    slabs = _make_slabs(np.asarray(displacement_field))
    band = _make_band()
    in_maps = [{"slab": s, "band": band} for s in slabs]
    res = run_bass_kernel_spmd(nc, in_maps, core_ids=list(range(N_CORES)))

    mask = _valid_mask()
    total = 0.0
    for k in range(N_CORES):
        acc = res.results[k]["acc"]          # [3, 128, NBLK] f32
        total += acc[mask].sum(dtype=np.float64)
    loss = total / float(B * D * H * W)
    return np.float32(loss)


if __name__ == "__main__":
    u = np.load('/root/problem/u_input.npy')
    print("loss:", kernel(u))



# revision 5
# speedup vs baseline: 1.3025x; 1.3025x over previous
"""JacobianDeterminantLoss Trainium2 kernel (8-core SPMD).

Math: u [2,3,160,192,160] f32 -> loss = mean(relu(-det(J))) where
J = I + grad(phi), phi_c = u_c * (dim_c-1)/2, gradients np.gradient
style (central interior, one-sided edges; ghosts 2a-b make both
uniform central diffs).

Layout (per core): core = (batch b, D-quarter q). Partitions =
3 H-slots x 42 planes (40 real + 1 halo each side) = 126. Per
partition free = 66 stored h-rows (64 real + halo) x 162 cols
(160 + ghost) fp16.

Host folds everything linear into the input: phi' = u*scale/2 +
0.5*(axis_idx - center) per channel. The central diff of the ramp
is exactly the +1 diagonal of J, so the device never adds 1.

Device per 12-row chunk:
- PE: block-diag band matmul -> D-diffs A,d,g in PSUM f32.
- ACT: PSUM -> SBUF fp16 copies; final relu + free-dim accum.
- DVE: shifted-diff subs + cofactor products (fp16 2x mode).
- Pool(GPSIMD): 2 diffs + the 3 (A,d,g)*minor products as
  scalar_tensor_tensor (its cheapest op class in the cost model).
Host: mask halo partitions, sum, divide by N.
"""
import sys
import numpy as np

if '/opt/trn_rl_repo' not in sys.path:
    sys.path.insert(0, '/opt/trn_rl_repo')

B, C, D, H, W = 2, 3, 160, 192, 160
N_CORES = 8
QP = D // 4                  # 40 planes per quarter
SLOT = QP + 2                # 42 partitions per slot
NSLOT = 3
NPART = NSLOT * SLOT         # 126
RS = H // NSLOT              # 64 real rows per slot
RSTORE = RS + 2              # 66 stored rows
WG = W + 2                   # 162 stored cols
CHUNKS = [(0, 12), (12, 12), (24, 12), (36, 12), (48, 12), (60, 4)]
NCHUNK = len(CHUNKS)
# DMA row sections (cover chunk c + halo before chunk c computes)
SECS = [(0, 14), (14, 13), (27, 13), (40, 13), (53, 13)]

_prog_cache = {}


def _build_program():
    import concourse.tile as tile
    import concourse.mybir as mybir
    from concourse import bacc

    fp16 = mybir.dt.float16
    f32 = mybir.dt.float32
    AT = mybir.AluOpType
    AF = mybir.ActivationFunctionType

    nc = bacc.Bacc("TRN2", target_bir_lowering=False, debug=False,
                   num_devices=N_CORES)
    slab_in = nc.dram_tensor("slab", [C, NPART, RSTORE, WG], fp16,
                             kind="ExternalInput")
    band_in = nc.dram_tensor("band", [128, 128], fp16, kind="ExternalInput")
    acc_out = nc.dram_tensor("acc", [NPART, NCHUNK], f32,
                             kind="ExternalOutput")

    with tile.TileContext(nc) as tc:
        with tc.tile_pool(name="inp", bufs=1) as inp, \
             tc.tile_pool(name="piece", bufs=2) as piece, \
             tc.tile_pool(name="dveonly", bufs=1) as dv, \
             tc.tile_pool(name="cross", bufs=2) as cx, \
             tc.tile_pool(name="misc", bufs=1) as misc, \
             tc.tile_pool(name="psum", bufs=1, space="PSUM") as psum:
            band = misc.tile([128, 128], fp16)
            nc.sync.dma_start(band[:], band_in[:])
            acc_sb = misc.tile([128, NCHUNK], f32)

            XYZ = []
            for c in range(C):
                t = inp.tile([128, RSTORE, WG], fp16, tag=f"in{c}")
                XYZ.append(t)
            for (s0, sn) in SECS:
                for c in range(C):
                    nc.sync.dma_start(XYZ[c][0:NPART, s0:s0 + sn],
                                      slab_in[c, :, s0:s0 + sn])
            X, Y, Z = XYZ

            for ci, (r0, nr) in enumerate(CHUNKS):
                # ---- PE: D-diffs -> PSUM, ACT: -> SBUF fp16 pieces ----
                pcs = []
                for ch in range(C):
                    pc = piece.tile([128, 12, W], fp16, tag=f"pc{ch}")
                    for g0 in range(0, nr, 6):
                        gn = min(6, nr - g0)
                        hb = (gn + 1) // 2
                        ps = psum.tile([128, 2, 512], f32, tag=f"ps{ch}")
                        for hh in range(2):
                            rr = g0 + hb * hh
                            rows = min(hb, gn - hb * hh)
                            if rows <= 0:
                                continue
                            nc.tensor.matmul(
                                ps[0:NPART, hh, 0:rows * W],
                                band[0:NPART, 0:NPART],
                                XYZ[ch][0:NPART, 1 + r0 + rr:1 + r0 + rr + rows,
                                        1:1 + W],
                                start=True, stop=True)
                        nc.scalar.copy(pc[0:NPART, g0:g0 + gn, :],
                                       ps[0:NPART, :, 0:hb * W])
                    pcs.append(pc)
                A_, d_, g_ = pcs

                def hv(t, dr):
                    return t[0:NPART, r0 + 1 + dr:r0 + 1 + dr + nr, 1:1 + W]

                def wv(t, dw):
                    return t[0:NPART, r0 + 1:r0 + 1 + nr, 1 + dw:1 + dw + W]

                def dtile(tag):
                    return dv.tile([128, 12, W], fp16, tag=tag, name=tag)

                def ctile(tag):
                    return cx.tile([128, 12, W], fp16, tag=tag, name=tag)

                # diffs: b on Pool; c,E,f,h_,I on DVE
                b_ = ctile("b")
                nc.gpsimd.tensor_sub(b_[0:NPART, 0:nr], hv(X, 1), hv(X, -1))
                c_ = ctile("c")
                nc.vector.tensor_sub(c_[0:NPART, 0:nr], wv(X, 1), wv(X, -1))
                E_ = dtile("E")
                nc.vector.tensor_sub(E_[0:NPART, 0:nr], hv(Y, 1), hv(Y, -1))
                f_ = dtile("f")
                nc.vector.tensor_sub(f_[0:NPART, 0:nr], wv(Y, 1), wv(Y, -1))
                h_ = dtile("h")
                nc.vector.tensor_sub(h_[0:NPART, 0:nr], hv(Z, 1), hv(Z, -1))
                I_ = dtile("i")
                nc.vector.tensor_sub(I_[0:NPART, 0:nr], wv(Z, 1), wv(Z, -1))

                def vb(t):
                    return t[0:NPART, 0:nr]

                # det = A*(EI - fh) - d*(bI - ch) + g*(bf - cE)
                p1 = dtile("p1")
                p2 = dtile("p2")
                M1 = ctile("M1")
                nc.vector.tensor_mul(vb(p1), vb(E_), vb(I_))
                nc.vector.tensor_mul(vb(p2), vb(f_), vb(h_))
                nc.vector.tensor_sub(vb(M1), vb(p1), vb(p2))
                M2 = ctile("M2")
                nc.vector.tensor_mul(vb(p1), vb(b_), vb(I_))
                nc.vector.tensor_mul(vb(p2), vb(c_), vb(h_))
                nc.vector.tensor_sub(vb(M2), vb(p1), vb(p2))
                M3 = ctile("M3")
                nc.vector.tensor_mul(vb(p1), vb(b_), vb(f_))
                nc.vector.tensor_mul(vb(p2), vb(c_), vb(E_))
                nc.vector.tensor_sub(vb(M3), vb(p1), vb(p2))

                T1 = ctile("T1")
                nc.gpsimd.tensor_mul(vb(T1), A_[0:NPART, 0:nr], vb(M1))
                T2 = ctile("T2")
                nc.gpsimd.tensor_mul(vb(T2), d_[0:NPART, 0:nr], vb(M2))
                T3 = ctile("T3")
                nc.gpsimd.tensor_mul(vb(T3), g_[0:NPART, 0:nr], vb(M3))

                n1 = dtile("n1")
                nc.vector.tensor_sub(vb(n1), vb(T2), vb(T1))
                nd = ctile("nd")
                nc.vector.tensor_sub(vb(nd), vb(n1), vb(T3))
                trash = ctile("trash")
                nc.scalar.activation(vb(trash), vb(nd), AF.Relu,
                                     accum_out=acc_sb[0:NPART, ci:ci + 1])
            nc.sync.dma_start(acc_out[:], acc_sb[0:NPART])
    nc.compile()
    return nc


def _make_band():
    band = np.zeros((128, 128), dtype=np.float16)
    for p in range(NPART):
        j = p % SLOT
        if j <= SLOT - 2:
            band[p + 1, p] = 1.0
        if j >= 1:
            band[p - 1, p] = -1.0
    return band


def _make_slabs(u):
    """u [2,3,160,192,160] -> 8 per-core slabs [3, 126, 66, 162] fp16."""
    u = np.asarray(u, dtype=np.float32)
    sc = np.array([(D - 1) / 4.0, (H - 1) / 4.0, (W - 1) / 4.0],
                  dtype=np.float32)
    phi = u * sc[None, :, None, None, None]
    # +1 diagonal as linear ramps (centered to limit fp16 magnitude)
    rd = 0.5 * (np.arange(D, dtype=np.float32) - (D - 1) / 2.0)
    rh = 0.5 * (np.arange(H, dtype=np.float32) - (H - 1) / 2.0)
    rw = 0.5 * (np.arange(W, dtype=np.float32) - (W - 1) / 2.0)
    phi[:, 0] += rd[:, None, None]
    phi[:, 1] += rh[None, :, None]
    phi[:, 2] += rw[None, None, :]
    # pad with linear-extrapolation ghosts on all three axes
    P = np.empty((B, C, D + 2, H + 2, W + 2), dtype=np.float32)
    P[:, :, 1:D + 1, 1:H + 1, 1:W + 1] = phi
    P[:, :, 1:D + 1, 1:H + 1, 0] = 2 * phi[..., 0] - phi[..., 1]
    P[:, :, 1:D + 1, 1:H + 1, W + 1] = 2 * phi[..., -1] - phi[..., -2]
    P[:, :, 1:D + 1, 0] = 2 * P[:, :, 1:D + 1, 1] - P[:, :, 1:D + 1, 2]
    P[:, :, 1:D + 1, H + 1] = 2 * P[:, :, 1:D + 1, H] - P[:, :, 1:D + 1, H - 1]
    P[:, :, 0] = 2 * P[:, :, 1] - P[:, :, 2]
    P[:, :, D + 1] = 2 * P[:, :, D] - P[:, :, D - 1]
    P16 = P.astype(np.float16)
    slabs = []
    for b in range(B):
        for q in range(4):
            # slot s, j: plane 40q-1+j -> padded idx 40q+j; row 64s-1+r -> 64s+r
            blocks = [P16[b, :, QP * q:QP * q + SLOT, RS * s:RS * s + RSTORE, :]
                      for s in range(NSLOT)]
            slab = np.concatenate(blocks, axis=1)  # [C, 126, 66, 162]
            slabs.append(np.ascontiguousarray(slab))
    return slabs


def _valid_mask():
    j = np.arange(NPART) % SLOT
    return (j >= 1) & (j <= SLOT - 2)


def kernel(displacement_field: np.ndarray) -> np.ndarray:
    from concourse.bass_utils import run_bass_kernel_spmd

    if 'nc' not in _prog_cache:
        _prog_cache['nc'] = _build_program()
    nc = _prog_cache['nc']

    slabs = _make_slabs(displacement_field)
    band = _make_band()
    in_maps = [{"slab": s, "band": band} for s in slabs]
    res = run_bass_kernel_spmd(nc, in_maps, core_ids=list(range(N_CORES)))

    mask = _valid_mask()
    total = 0.0
    for k in range(N_CORES):
        acc = res.results[k]["acc"]          # [126, NCHUNK] f32
        total += acc[mask].sum(dtype=np.float64)
    loss = total / float(B * D * H * W)
    return np.float32(loss)


if __name__ == "__main__":
    u = np.load('/root/problem/u_input.npy')
    print("loss:", kernel(u))


# revision 6
# speedup vs baseline: 1.5353x; 1.1788x over previous
"""JacobianDeterminantLoss Trainium2 kernel (8-core SPMD).

Math: u [2,3,160,192,160] f32 -> loss = mean(relu(-det(J))) where
J = I + grad(phi), phi_c = u_c * (dim_c-1)/2, gradients np.gradient
style (central interior, one-sided edges; ghosts 2a-b make both
uniform central diffs).

Layout (per core): core = (batch b, D-quarter q). Partitions =
3 H-slots x 42 planes (40 real + 1 halo each side) = 126. Per
partition free = 66 stored h-rows (64 real + halo) x 162 cols
(160 + ghost) fp16.

Host folds everything linear into the input: phi' = u*scale/2 +
0.5*(axis_idx - center) per channel. The central diff of the ramp
is exactly the +1 diagonal of J, so the device never adds 1.

Device per 12-row chunk:
- PE: block-diag band matmul -> D-diffs A,d,g in PSUM f32.
- ACT: PSUM -> SBUF fp16 copies; final relu + free-dim accum.
- DVE: shifted-diff subs + cofactor products (fp16 2x mode).
- Pool(GPSIMD): 2 diffs + the 3 (A,d,g)*minor products as
  scalar_tensor_tensor (its cheapest op class in the cost model).
Host: mask halo partitions, sum, divide by N.
"""
import sys
import numpy as np

if '/opt/trn_rl_repo' not in sys.path:
    sys.path.insert(0, '/opt/trn_rl_repo')

B, C, D, H, W = 2, 3, 160, 192, 160
N_CORES = 8
QP = D // 4                  # 40 planes per quarter
SLOT = QP + 2                # 42 partitions per slot
NSLOT = 3
NPART = NSLOT * SLOT         # 126
RS = H // NSLOT              # 64 real rows per slot
RSTORE = RS + 2              # 66 stored rows
WG = W + 2                   # 162 stored cols
CHUNKS = [(0, 12), (12, 12), (24, 12), (36, 12), (48, 12), (60, 4)]
NCHUNK = len(CHUNKS)
# DMA row sections (cover chunk c + halo before chunk c computes)
SECS = [(0, 14), (14, 13), (27, 13), (40, 13), (53, 13)]

_prog_cache = {}


def _build_program():
    import concourse.tile as tile
    import concourse.mybir as mybir
    from concourse import bacc

    fp16 = mybir.dt.float16
    f32 = mybir.dt.float32
    AT = mybir.AluOpType
    AF = mybir.ActivationFunctionType

    nc = bacc.Bacc("TRN2", target_bir_lowering=False, debug=False,
                   num_devices=N_CORES)
    slab_in = nc.dram_tensor("slab", [C, NPART, RSTORE, WG], fp16,
                             kind="ExternalInput")
    band_in = nc.dram_tensor("band", [128, 128], fp16, kind="ExternalInput")
    acc_out = nc.dram_tensor("acc", [NPART, NCHUNK], f32,
                             kind="ExternalOutput")

    with tile.TileContext(nc) as tc:
        with tc.tile_pool(name="inp", bufs=1) as inp, \
             tc.tile_pool(name="piece", bufs=2) as piece, \
             tc.tile_pool(name="dveonly", bufs=1) as dv, \
             tc.tile_pool(name="cross", bufs=2) as cx, \
             tc.tile_pool(name="misc", bufs=1) as misc, \
             tc.tile_pool(name="psum", bufs=1, space="PSUM") as psum:
            band = misc.tile([128, 128], fp16)
            nc.sync.dma_start(band[:], band_in[:])
            acc_sb = misc.tile([128, NCHUNK], f32)

            XYZ = []
            for c in range(C):
                t = inp.tile([128, RSTORE, WG], fp16, tag=f"in{c}")
                XYZ.append(t)
            for (s0, sn) in SECS:
                for c in range(C):
                    nc.sync.dma_start(XYZ[c][0:NPART, s0:s0 + sn],
                                      slab_in[c, :, s0:s0 + sn])
            X, Y, Z = XYZ

            def stage1(ci):
                """PE D-diffs + ACT copies + diffs/products/minors."""
                r0, nr = CHUNKS[ci]
                pcs = []
                for ch in range(C):
                    pc = piece.tile([128, 12, W], fp16, tag=f"pc{ch}",
                                    name=f"pc{ch}")
                    for g0 in range(0, nr, 6):
                        gn = min(6, nr - g0)
                        hb = (gn + 1) // 2
                        ps = psum.tile([128, 2, 512], f32, tag=f"ps{ch}",
                                       name=f"ps{ch}")
                        for hh in range(2):
                            rr = g0 + hb * hh
                            rows = min(hb, gn - hb * hh)
                            if rows <= 0:
                                continue
                            nc.tensor.matmul(
                                ps[0:NPART, hh, 0:rows * W],
                                band[0:NPART, 0:NPART],
                                XYZ[ch][0:NPART, 1 + r0 + rr:1 + r0 + rr + rows,
                                        1:1 + W],
                                start=True, stop=True)
                        nc.scalar.copy(pc[0:NPART, g0:g0 + gn, :],
                                       ps[0:NPART, :, 0:hb * W])
                    pcs.append(pc)

                def hv(t, dr):
                    return t[0:NPART, r0 + 1 + dr:r0 + 1 + dr + nr, 1:1 + W]

                def wv(t, dw):
                    return t[0:NPART, r0 + 1:r0 + 1 + nr, 1 + dw:1 + dw + W]

                def dtile(tag):
                    return dv.tile([128, 12, W], fp16, tag=tag, name=tag)

                def ctile(tag):
                    return cx.tile([128, 12, W], fp16, tag=tag, name=tag)

                def vb(t):
                    return t[0:NPART, 0:nr]

                # diffs: b on Pool; c,E,f,h_,I on DVE
                b_ = ctile("b")
                nc.gpsimd.tensor_sub(vb(b_), hv(X, 1), hv(X, -1))
                c_ = dtile("c")
                nc.vector.tensor_sub(vb(c_), wv(X, 1), wv(X, -1))
                E_ = dtile("E")
                nc.vector.tensor_sub(vb(E_), hv(Y, 1), hv(Y, -1))
                f_ = dtile("f")
                nc.vector.tensor_sub(vb(f_), wv(Y, 1), wv(Y, -1))
                h_ = dtile("h")
                nc.vector.tensor_sub(vb(h_), hv(Z, 1), hv(Z, -1))
                I_ = dtile("i")
                nc.vector.tensor_sub(vb(I_), wv(Z, 1), wv(Z, -1))

                # det = A*(EI - fh) - d*(bI - ch) + g*(bf - cE)
                p1 = dtile("p1")
                p2 = dtile("p2")
                M1 = ctile("M1")
                nc.vector.tensor_mul(vb(p1), vb(E_), vb(I_))
                nc.vector.tensor_mul(vb(p2), vb(f_), vb(h_))
                nc.vector.tensor_sub(vb(M1), vb(p1), vb(p2))
                M2 = ctile("M2")
                nc.vector.tensor_mul(vb(p1), vb(b_), vb(I_))
                nc.vector.tensor_mul(vb(p2), vb(c_), vb(h_))
                nc.vector.tensor_sub(vb(M2), vb(p1), vb(p2))
                M3 = ctile("M3")
                nc.vector.tensor_mul(vb(p1), vb(b_), vb(f_))
                nc.vector.tensor_mul(vb(p2), vb(c_), vb(E_))
                nc.vector.tensor_sub(vb(M3), vb(p1), vb(p2))
                return pcs, (M1, M2, M3)

            def stage2(ci, pcs, Ms):
                r0, nr = CHUNKS[ci]
                A_, d_, g_ = pcs
                M1, M2, M3 = Ms

                def dtile(tag):
                    return dv.tile([128, 12, W], fp16, tag=tag, name=tag)

                def ctile(tag):
                    return cx.tile([128, 12, W], fp16, tag=tag, name=tag)

                def vb(t):
                    return t[0:NPART, 0:nr]

                T1 = ctile("T1")
                nc.gpsimd.tensor_mul(vb(T1), A_[0:NPART, 0:nr], vb(M1))
                T2 = ctile("T2")
                nc.gpsimd.tensor_mul(vb(T2), d_[0:NPART, 0:nr], vb(M2))
                T3 = ctile("T3")
                nc.gpsimd.tensor_mul(vb(T3), g_[0:NPART, 0:nr], vb(M3))
                n1 = dtile("n1")
                nc.vector.tensor_sub(vb(n1), vb(T2), vb(T1))
                nd = ctile("nd")
                nc.vector.tensor_sub(vb(nd), vb(n1), vb(T3))
                trash = ctile("trash")
                nc.scalar.activation(vb(trash), vb(nd), AF.Relu,
                                     accum_out=acc_sb[0:NPART, ci:ci + 1])

            pending = None
            for ci in range(NCHUNK):
                s1 = stage1(ci)
                if pending is not None:
                    stage2(ci - 1, *pending)
                pending = s1
            stage2(NCHUNK - 1, *pending)
            nc.sync.dma_start(acc_out[:], acc_sb[0:NPART])
    nc.compile()
    return nc


def _make_band():
    band = np.zeros((128, 128), dtype=np.float16)
    for p in range(NPART):
        j = p % SLOT
        if j <= SLOT - 2:
            band[p + 1, p] = 1.0
        if j >= 1:
            band[p - 1, p] = -1.0
    return band


def _make_slabs(u):
    """u [2,3,160,192,160] -> 8 per-core slabs [3, 126, 66, 162] fp16."""
    u = np.asarray(u, dtype=np.float32)
    sc = np.array([(D - 1) / 4.0, (H - 1) / 4.0, (W - 1) / 4.0],
                  dtype=np.float32)
    phi = u * sc[None, :, None, None, None]
    # +1 diagonal as linear ramps (centered to limit fp16 magnitude)
    rd = 0.5 * (np.arange(D, dtype=np.float32) - (D - 1) / 2.0)
    rh = 0.5 * (np.arange(H, dtype=np.float32) - (H - 1) / 2.0)
    rw = 0.5 * (np.arange(W, dtype=np.float32) - (W - 1) / 2.0)
    phi[:, 0] += rd[:, None, None]
    phi[:, 1] += rh[None, :, None]
    phi[:, 2] += rw[None, None, :]
    # pad with linear-extrapolation ghosts on all three axes
    P = np.empty((B, C, D + 2, H + 2, W + 2), dtype=np.float32)
    P[:, :, 1:D + 1, 1:H + 1, 1:W + 1] = phi
    P[:, :, 1:D + 1, 1:H + 1, 0] = 2 * phi[..., 0] - phi[..., 1]
    P[:, :, 1:D + 1, 1:H + 1, W + 1] = 2 * phi[..., -1] - phi[..., -2]
    P[:, :, 1:D + 1, 0] = 2 * P[:, :, 1:D + 1, 1] - P[:, :, 1:D + 1, 2]
    P[:, :, 1:D + 1, H + 1] = 2 * P[:, :, 1:D + 1, H] - P[:, :, 1:D + 1, H - 1]
    P[:, :, 0] = 2 * P[:, :, 1] - P[:, :, 2]
    P[:, :, D + 1] = 2 * P[:, :, D] - P[:, :, D - 1]
    P16 = P.astype(np.float16)
    slabs = []
    for b in range(B):
        for q in range(4):
            # slot s, j: plane 40q-1+j -> padded idx 40q+j; row 64s-1+r -> 64s+r
            blocks = [P16[b, :, QP * q:QP * q + SLOT, RS * s:RS * s + RSTORE, :]
                      for s in range(NSLOT)]
            slab = np.concatenate(blocks, axis=1)  # [C, 126, 66, 162]
            slabs.append(np.ascontiguousarray(slab))
    return slabs


def _valid_mask():
    j = np.arange(NPART) % SLOT
    return (j >= 1) & (j <= SLOT - 2)


def kernel(displacement_field: np.ndarray) -> np.ndarray:
    from concourse.bass_utils import run_bass_kernel_spmd

    if 'nc' not in _prog_cache:
        _prog_cache['nc'] = _build_program()
    nc = _prog_cache['nc']

    slabs = _make_slabs(displacement_field)
    band = _make_band()
    in_maps = [{"slab": s, "band": band} for s in slabs]
    res = run_bass_kernel_spmd(nc, in_maps, core_ids=list(range(N_CORES)))

    mask = _valid_mask()
    total = 0.0
    for k in range(N_CORES):
        acc = res.results[k]["acc"]          # [126, NCHUNK] f32
        total += acc[mask].sum(dtype=np.float64)
    loss = total / float(B * D * H * W)
    return np.float32(loss)


if __name__ == "__main__":
    u = np.load('/root/problem/u_input.npy')
    print("loss:", kernel(u))
